# revision 25
# baseline (speedup 1.0000x reference)
"""DoRA embedding kernel for 8 Trainium2 NeuronCores.

Math (reference):
    C = E + s * A @ B                  # [V, D]
    n = max(||C||_col, 1e-8)           # [D]
    out = (C / n * mag)[token_ids]     # [B, S, D]

Strategy: shard D=768 columns across 8 cores (96 cols each), no collectives.

Pass 1 (norms) — self-Gram on PE, fp8:
    Stream vocab-major tiles T = [256*E_cols | 256*A] (fp8, [128v, 112], four
    vocab rows packed per 512B table row for full-rate DMA) through
    matmul(T^T T), accumulating G = [[E'E', E'A'],[A'E', A'A']] in one PSUM
    bank (396 matmuls, one accumulation group).  Then
        sumsq*65536 = diag(E'E') + sum_r (sB (.) (2M^T + A'A' sB))
    via small matmuls operating directly on Gram slices at partitions 96:112
    (tile_position row offset 96 — no SBUF repack DMA), rsqrt via DVE
    reciprocal + one Newton step, giving sclT = mag/||C||_col in [1, 96].

Pass 2 (lookup) — TRANSPOSED parity-bucketed gathers + one fused matmul:
    dma_gather(transpose=True) lands each 1024-token chunk feature-major:
    g[p, 0, t] = row-element p of token t, i.e. partitions 0:96 = E_c^T,
    96:112 = A^T.  The entire (E + A sB) * s is then ONE matmul per
    512-token block with the fused stationary W2 = [[diag(s)], [s (.) sB]]
    ([112, 96] bf16, built on device from sclT):
        psum[col, tok] = sum_k W2[k, col] g[k, tok]
                       = s_col E^T[col, tok] + sum_r s_col sB[r, col] A^T[r, tok].
    DVE/ACT alternate evacuating psum -> bf16 [96, 1024] tiles, DMA out
    column-major [96, nch*1024]; host transposes/un-permutes/upcasts.
"""

import sys
from contextlib import ExitStack

import numpy as np

for _p in ("/opt/trn_rl_repo",):
    if _p not in sys.path:
        sys.path.append(_p)

import ml_dtypes
import concourse.bass as bass
import concourse.bacc as bacc
import concourse.tile as tile
from concourse import mybir, bass_utils

F32 = mybir.dt.float32
BF16 = mybir.dt.bfloat16
FP8 = mybir.dt.float8e4
I16 = mybir.dt.int16
ALU = mybir.AluOpType
ACTF = mybir.ActivationFunctionType

V, D, R = 50257, 768, 16
SCALING = 32.0 / 16.0
N_CORES = 8
CPC = D // N_CORES          # columns per core = 96
W = CPC + R                 # 112 packed feature width
EF8 = 256.0                 # fp8 pre-scale for E and A in pass 1
VP = 50688                  # vocab padded: 99 groups of 512 rows
NG = VP // 512              # 99 row-groups (4 matmuls each)
GL = 9                      # groups per DMA macro-load (99 = 11 * 9)
PAIRS = ((V + 1) // 2 + 127) // 128 * 128  # 25216 pair rows
NTOK = 8 * 2048
CHUNK = 2048                # max tokens per dma_gather
HB = 512                    # tokens per matmul block (one PSUM bank)


def _chunk_sizes(n):
    """Split one parity bucket into gather chunks: multiples of HB, <=CHUNK."""
    out = []
    while n > 0:
        take = min(CHUNK, (min(n, CHUNK) + HB - 1) // HB * HB)
        out.append(take)
        n -= take
    return out


def _apply_drain_patch():
    """walrus in this container rejects >1 sem-wait on the Tile tail drain
    ("Too many sync wait commands"); split the waits across chained drains."""
    import concourse.tile as _tile_mod
    if getattr(_tile_mod.TileContext, "_drain_patch_applied", False):
        return

    def _drain_and_barrier(self, tick_clock, wait_clock):
        from concourse.tile import ScopedClock

        nc = self.nc
        drain_inst = nc.sync.drain()
        wait_clock.add_sem_waits(
            drain_inst.ins, ScopedClock({None: tick_clock.global_clock})
        )
        si = drain_inst.ins.sync_info
        if si is not None and si.on_wait and len(si.on_wait) > 1:
            waits = list(si.on_wait)
            del si.on_wait[1:]
            for w in waits[1:]:
                extra = nc.sync.drain()
                esi = extra.ins.sync_info
                if esi is None:
                    extra.ins.sync_info = mybir.SyncInfo(on_wait=[w], on_update=[])
                else:
                    esi.on_wait.append(w)
        nc.all_engine_barrier()
        assert self.sems is not None
        popped = nc._tile_sem_poison_stack.pop()
        assert popped is self._sem_poison
        nc.clear_and_free_semaphores(list(self.sems.allocated().values()))
        nc.all_engine_barrier()

    _tile_mod.TileContext._drain_and_barrier = _drain_and_barrier
    _tile_mod.TileContext._drain_patch_applied = True


_CACHED = {}
NREP = 1
VARIANT = "full"


def _build(nch):
    chunks = tuple(_build.chunk_parity)   # [(size, parity), ...]
    T = sum(s for s, _ in chunks)
    key = (NREP, VARIANT, chunks)
    if key in _CACHED:
        return _CACHED[key]
    _apply_drain_patch()

    nc = bacc.Bacc("TRN2", target_bir_lowering=False, debug=False)
    d_p1t = nc.dram_tensor("p1t", [VP // 4, 512], FP8, kind="ExternalInput").ap()
    d_gtab = nc.dram_tensor("gtab", [PAIRS, 256], BF16, kind="ExternalInput").ap()
    # sb96: rows 96:112 hold SCALING*B[:, cols] (f32); others zero
    d_sb96 = nc.dram_tensor("sb96", [128, CPC], F32, kind="ExternalInput").ap()
    # w2c: unscaled fused pass-2 stationary [[I_96], [SCALING*B]] bf16
    d_w2c = nc.dram_tensor("w2c", [W, CPC], BF16, kind="ExternalInput").ap()
    d_mag = nc.dram_tensor("magT", [1, CPC], F32, kind="ExternalInput").ap()
    d_idf = nc.dram_tensor("identf", [128, 128], F32, kind="ExternalInput").ap()
    d_i96 = nc.dram_tensor("i96", [CPC, CPC], F32, kind="ExternalInput").ap()
    # twoi96: rows 96:112 hold 2*I_16; others zero
    d_twoi = nc.dram_tensor("twoi96", [128, R], F32, kind="ExternalInput").ap()
    # ones16_96: rows 96:112 hold 1.0; others zero
    d_ones16 = nc.dram_tensor("ones16_96", [128, 1], F32, kind="ExternalInput").ap()
    d_ones = nc.dram_tensor("ones1", [1, 128], F32, kind="ExternalInput").ap()
    d_pidx = nc.dram_tensor("pidx", [128, T // 16], I16,
                            kind="ExternalInput").ap()
    # column-major output: padded-stream token j at column j, rows = 96 cols
    d_out = nc.dram_tensor("out", [CPC, T], BF16, kind="ExternalOutput").ap()

    with tile.TileContext(nc) as tc, ExitStack() as ctx:
        const = ctx.enter_context(tc.tile_pool(name="const", bufs=1))

        sb96_sb = const.tile([128, CPC], F32)
        nc.sync.dma_start(out=sb96_sb, in_=d_sb96)
        w2c_sb = const.tile([W, CPC], BF16)
        nc.sync.dma_start(out=w2c_sb, in_=d_w2c)
        mag_sb = const.tile([1, CPC], F32)
        nc.sync.dma_start(out=mag_sb, in_=d_mag)
        idf_sb = const.tile([128, 128], F32)
        nc.sync.dma_start(out=idf_sb, in_=d_idf)
        i96_sb = const.tile([CPC, CPC], F32)
        nc.sync.dma_start(out=i96_sb, in_=d_i96)
        twoi_sb = const.tile([128, R], F32)
        nc.sync.dma_start(out=twoi_sb, in_=d_twoi)
        ones16_sb = const.tile([128, 1], F32)
        nc.sync.dma_start(out=ones16_sb, in_=d_ones16)
        ones_sb = const.tile([1, 128], F32)
        nc.sync.dma_start(out=ones_sb, in_=d_ones)
        pidx_sb = const.tile([128, T // 16], I16)
        nc.sync.dma_start(out=pidx_sb, in_=d_pidx)

    # number of evacuations routed to ACT vs DVE (interleave by index)

        def _dummy_out(rep_pool, src=None):
            outt0 = rep_pool.tile([CPC, 8], BF16)
            nc.vector.memset(outt0, 0.0)
            if src is not None:
                nc.vector.tensor_copy(out=outt0[0:1, 0:1], in_=src)
            nc.sync.dma_start(out=d_out[0:CPC, 0:8], in_=outt0)

        def _emit(chunk_parity, rep_pool, p1l, p1ps, sc, scps, p2g, p2ps, p2o):
            if VARIANT == "nop":
                _dummy_out(rep_pool)
                return

            # ---- pass 2 gathers: transposed 256B rows (issue first) ----
            # single_packet=True overflows the 64-desc/packet ring limit at
            # num_idxs>=1024 on the transpose path (device-fatal); keep False.
            gt = []
            n_gather = {"p2g1": 1}.get(VARIANT, len(chunks))
            if VARIANT not in ("p1", "p1mm", "p1dma"):
                coff = 0
                for c in range(n_gather):
                    size, par = chunks[c]
                    g = p2g.tile([128, 1, CHUNK], BF16, tag="g")
                    nc.gpsimd.dma_gather(
                        g[:, :, 0:size],
                        d_gtab[:, 128 * par : 128 * par + 128],
                        pidx_sb[:, coff // 16 : (coff + size) // 16],
                        num_idxs=size,
                        num_idxs_reg=size,
                        elem_size=128,
                        elem_step=256,
                        transpose=True,
                        single_packet=False,
                    )
                    gt.append(g)
                    coff += size
            if VARIANT in ("p2g", "p2g1"):
                outt0 = rep_pool.tile([CPC, 128], BF16)
                nc.scalar.copy(out=outt0, in_=gt[0][0:CPC, 0, 0:128])
                nc.sync.dma_start(out=d_out[0:CPC, 0:128], in_=outt0)
                return

            # ---- pass 1: self-Gram over vocab ----
            if VARIANT == "p2":
                sclv = sc.tile([CPC, 1], F32, tag="sclv")
                nc.vector.memset(sclv, 1.0)
            else:
                gram = p1ps.tile([W, W], F32, tag="gram")
                n_mm = NG * 4
                k = 0
                for i in range(NG // GL):
                    t = p1l.tile([128, GL, 4, 128], FP8)
                    nc.sync.dma_start(
                        out=t,
                        in_=d_p1t[i * GL * 128 : (i + 1) * GL * 128, :].rearrange(
                            "(g p) (j w) -> p g j w", p=128, j=4
                        ),
                    )
                    if VARIANT == "p1dma":
                        continue
                    for g in range(GL):
                        for j in range(4):
                            nc.tensor.matmul(
                                gram,
                                t[:, g, j, 0:W],
                                t[:, g, j, 0:W],
                                start=(k == 0),
                                stop=(k == n_mm - 1),
                            )
                            k += 1
                if VARIANT in ("p1mm", "p1dma"):
                    outt0 = rep_pool.tile([CPC, 8], BF16)
                    if VARIANT == "p1mm":
                        nc.scalar.copy(out=outt0, in_=gram[0:CPC, 0:8])
                    else:
                        nc.vector.memset(outt0, 0.0)
                    nc.sync.dma_start(out=d_out[0:CPC, 0:8], in_=outt0)
                    return

                # ---- sumsq (x65536) from Gram pieces ----
                gram_sb = sc.tile([W, W], F32, tag="gram_sb")
                nc.vector.tensor_copy(out=gram_sb, in_=gram)
                # t1[96, 1] = diag(E'E')
                dd = sc.tile([CPC, CPC], F32, tag="dd")
                nc.vector.tensor_tensor(
                    out=dd, in0=gram_sb[0:CPC, 0:CPC], in1=i96_sb, op=ALU.mult
                )
                t1 = sc.tile([CPC, 1], F32)
                nc.vector.reduce_sum(out=t1, in_=dd, axis=mybir.AxisListType.X)
                # u[16@96, 96] = A'A' @ sB + 2I @ M^T  (Gram slices in place)
                u_ps = scps.tile([128, CPC], F32, tag="chain")
                nc.tensor.matmul(u_ps[96:W, :], gram_sb[CPC:W, CPC:W],
                                 sb96_sb[96:W, :], start=True, stop=False,
                                 tile_position=(96, 96))
                nc.tensor.matmul(u_ps[96:W, :], twoi_sb[96:W, :],
                                 gram_sb[CPC:W, 0:CPC], start=False, stop=True,
                                 tile_position=(96, 96))
                vsb = sc.tile([128, CPC], F32, tag="vsb")
                nc.vector.tensor_tensor(out=vsb[96:W, :], in0=u_ps[96:W, :],
                                        in1=sb96_sb[96:W, :], op=ALU.mult)
                # ssT[1, 96] = ones16^T @ vsb + t1^T
                red_ps = scps.tile([128, CPC], F32, tag="chain")
                nc.tensor.matmul(red_ps[0:1, :], ones16_sb[96:W, :],
                                 vsb[96:W, :], start=True, stop=False,
                                 tile_position=(96, 0))
                nc.tensor.matmul(red_ps[0:1, :], t1, idf_sb[:CPC, :CPC],
                                 start=False, stop=True, is_transpose=True)
                ssT = sc.tile([1, CPC], F32)
                nc.vector.tensor_copy(out=ssT, in_=red_ps[0:1, :])

                # ---- sclT = 256*mag * rsqrt(ssT)  [1, 96] ----
                nrm = sc.tile([1, CPC], F32)
                nc.scalar.activation(nrm, ssT, ACTF.Sqrt)
                nc.vector.tensor_scalar(
                    out=nrm, in0=nrm, scalar1=EF8 * 1e-8, scalar2=None,
                    op0=ALU.max,
                )
                r0 = sc.tile([1, CPC], F32)
                nc.vector.reciprocal(out=r0, in_=nrm)
                sclT = sc.tile([1, CPC], F32)
                nc.vector.tensor_tensor(out=sclT, in0=r0, in1=mag_sb,
                                        op=ALU.mult)
                # transpose to per-partition [96, 1] for the evacuation scale
                tp_ps = scps.tile([128, CPC], F32, tag="chain")
                nc.tensor.matmul(tp_ps[0:CPC, 0:1], sclT, ones_sb[0:1, 0:1],
                                 is_transpose=True)
                sclv = sc.tile([CPC, 1], F32, tag="sclv")
                nc.vector.tensor_copy(out=sclv, in_=tp_ps[0:CPC, 0:1])

            if VARIANT == "p1":
                _dummy_out(rep_pool, sclv[0:1, 0:1])
                return

            # ---- pass 2: one fused matmul per 512-token block ----
            sclv_bc = bass.AP(
                tensor=sclv.tensor, offset=sclv.offset,
                ap=[list(sclv.ap[0]), [0, HB]],
            )
            coff = 0
            blk = 0
            for c, (size, _par) in enumerate(chunks):
                g = gt[c]
                nb = size // HB
                ot = p2o.tile([CPC, CHUNK // HB, HB], BF16, tag="ot")
                for h in range(nb):
                    ps = p2ps.tile([128, HB], F32, tag="ps")
                    nc.tensor.matmul(
                        ps[0:CPC, :], w2c_sb, g[0:W, 0, h * HB : (h + 1) * HB],
                        start=True, stop=True,
                    )
                    if blk % 2 == 0:
                        nc.vector.tensor_tensor(
                            out=ot[:, h, :], in0=ps[0:CPC, :], in1=sclv_bc,
                            op=ALU.mult,
                        )
                    else:
                        nc.scalar.activation(
                            ot[:, h, :], ps[0:CPC, :], ACTF.Copy, scale=sclv,
                        )
                    blk += 1
                nc.sync.dma_start(
                    out=d_out[:, coff : coff + size].rearrange(
                        "p (h f) -> p h f", h=nb
                    ),
                    in_=ot[:, 0:nb, :],
                )
                coff += size

        chunk_parity = _build.chunk_parity
        for _rep in range(NREP):
            if _rep:
                tc.strict_bb_all_engine_barrier()
            with (
                tc.tile_pool(name=f"rep{_rep}", bufs=1) as rep_pool,
                tc.tile_pool(name=f"p1l{_rep}", bufs=3) as p1l,
                tc.tile_pool(name=f"p1ps{_rep}", bufs=1, space="PSUM") as p1ps,
                tc.tile_pool(name=f"sc{_rep}", bufs=1) as sc,
                tc.tile_pool(name=f"scps{_rep}", bufs=1, space="PSUM") as scps,
                tc.tile_pool(name=f"p2g{_rep}", bufs=max(2, len(chunks))) as p2g,
                tc.tile_pool(name=f"p2ps{_rep}", bufs=6, space="PSUM") as p2ps,
                tc.tile_pool(name=f"p2o{_rep}", bufs=3) as p2o,
            ):
                _emit(chunk_parity, rep_pool, p1l, p1ps, sc, scps, p2g,
                      p2ps, p2o)

    nc.compile()
    _CACHED[key] = nc
    return nc


_build.chunk_parity = []


def _host_prep(inputs, embeddings, lora_a, lora_b, magnitude):
    E = np.asarray(embeddings, np.float32)
    A = np.asarray(lora_a, np.float32)
    B = np.asarray(lora_b, np.float32)
    mag = np.asarray(magnitude, np.float32)
    ids = np.asarray(inputs).astype(np.int64).reshape(-1)

    # ---- token parity bucketing, variable-size gather chunks ----
    even_pos = np.flatnonzero((ids & 1) == 0)
    odd_pos = np.flatnonzero((ids & 1) == 1)
    ne, no = len(even_pos), len(odd_pos)
    ce, co = _chunk_sizes(ne), _chunk_sizes(no)
    chunks = [(s, 0) for s in ce] + [(s, 1) for s in co]
    Se = sum(ce)
    T = Se + sum(co)
    perm = np.concatenate([even_pos, odd_pos])
    pair_ids = np.zeros(T, np.int64)
    pair_ids[:ne] = ids[even_pos] // 2
    pair_ids[Se : Se + no] = ids[odd_pos] // 2
    pidx_np = np.tile(
        pair_ids.astype(np.int16).reshape(T // 16, 16).T, (8, 1)
    ).copy()

    # ---- pass-1 fp8 table: [E*256 | A*256 | pad] vocab-major ----
    fp8 = ml_dtypes.float8_e4m3
    EAf = np.zeros((VP, 128), np.float32)
    EAf[:V, CPC : CPC + R] = EF8 * A

    # ---- pass-2 bf16 pair table ----
    Epad = np.zeros((2 * PAIRS, D), np.float32)
    Epad[:V] = E
    Apad = np.zeros((2 * PAIRS, R), np.float32)
    Apad[:V] = A

    idf_np = np.eye(128, dtype=np.float32)
    i96_np = np.eye(CPC, dtype=np.float32)
    twoi_np = np.zeros((128, R), np.float32)
    twoi_np[96:W, :] = 2.0 * np.eye(R, dtype=np.float32)
    ones16_np = np.zeros((128, 1), np.float32)
    ones16_np[96:W, :] = 1.0
    ones1_np = np.ones((1, 128), np.float32)

    in_maps = []
    for c in range(N_CORES):
        cols = slice(CPC * c, CPC * (c + 1))
        EAf[:V, 0:CPC] = EF8 * E[:, cols]
        p1t = EAf.astype(fp8).reshape(VP // 4, 512)

        ec = Epad[:, cols]
        gtab = np.zeros((PAIRS, 256), dtype=ml_dtypes.bfloat16)
        gtab[:, 0:CPC] = ec[0::2]
        gtab[:, CPC : CPC + R] = Apad[0::2]
        gtab[:, 128 : 128 + CPC] = ec[1::2]
        gtab[:, 128 + CPC : 128 + CPC + R] = Apad[1::2]

        sb96 = np.zeros((128, CPC), np.float32)
        sb96[96:W, :] = SCALING * B[:, cols]
        w2c = np.zeros((W, CPC), np.float32)
        w2c[0:CPC, :] = i96_np
        w2c[96:W, :] = SCALING * B[:, cols]
        in_maps.append(
            {
                "p1t": p1t,
                "gtab": gtab,
                "sb96": sb96,
                "w2c": w2c.astype(ml_dtypes.bfloat16),
                "magT": np.ascontiguousarray(EF8 * mag[cols])[None, :],
                "identf": idf_np,
                "i96": i96_np,
                "twoi96": twoi_np,
                "ones16_96": ones16_np,
                "ones1": ones1_np,
                "pidx": pidx_np,
            }
        )
    return in_maps, perm, ne, no, Se, len(chunks), chunks


def kernel(inputs, embeddings, lora_a, lora_b, magnitude, _trace=False):
    in_maps, perm, ne, no, Se, nch, chunks = _host_prep(
        inputs, embeddings, lora_a, lora_b, magnitude
    )
    _build.chunk_parity = chunks
    nc = _build(nch)
    res = bass_utils.run_bass_kernel_spmd(
        nc, in_maps, core_ids=list(range(N_CORES)), trace=_trace
    )
    shp = np.asarray(inputs).shape
    out = np.empty((NTOK, D), np.float32)
    valid = np.concatenate([np.arange(ne), Se + np.arange(no)])
    for c in range(N_CORES):
        blk = np.asarray(res.results[c]["out"], dtype=np.float32)
        # [96, nch*1024] column-major -> token-ordered [nch*1024, 96]
        out[perm, CPC * c : CPC * (c + 1)] = blk[:, valid].T
    out = out.reshape(shp + (D,))
    if _trace:
        return out, res
    return out


# revision 27
# speedup vs baseline: 1.1185x; 1.1185x over previous
"""DoRA embedding kernel for 8 Trainium2 NeuronCores.

Math (reference):
    C = E + s * A @ B                  # [V, D]
    n = max(||C||_col, 1e-8)           # [D]
    out = (C / n * mag)[token_ids]     # [B, S, D]

Strategy: shard D=768 columns across 8 cores (96 cols each), no collectives.

Pass 1 (norms) — self-Gram on PE, fp8:
    Stream vocab-major tiles T = [256*E_cols | 256*A] (fp8, [128v, 112], four
    vocab rows packed per 512B table row for full-rate DMA) through
    matmul(T^T T), accumulating G = [[E'E', E'A'],[A'E', A'A']] in one PSUM
    bank (396 matmuls, one accumulation group).  Then
        sumsq*65536 = diag(E'E') + sum_r (sB (.) (2M^T + A'A' sB))
    via small matmuls operating directly on Gram slices at partitions 96:112
    (tile_position row offset 96 — no SBUF repack DMA), then ACT sqrt + DVE
    reciprocal and a PE transpose of the [1, 96] scale row into the
    per-partition [96, 1] vector sclv = mag/||C||_col.

Pass 2 (lookup) — TRANSPOSED parity-bucketed gathers + one fused matmul:
    dma_gather(transpose=True, single_packet=False) lands each <=2048-token
    chunk feature-major: g[p, 0, t] = row-element p of token t, i.e.
    partitions 0:96 = E_c^T, 96:112 = A^T.  The whole E + A sB is then ONE
    matmul per 512-token block with the constant stationary
    W2 = [[I_96], [SCALING*B]] ([112, 96] bf16, host input — pass-2 matmuls
    depend only on the gathers, so they pipeline behind the Gram stream):
        psum[col, tok] = E^T[col, tok] + sum_r sB[r, col] A^T[r, tok].
    DVE (tensor_tensor, sclv broadcast) and ACT (activation Copy with
    per-partition scale=sclv) alternate evacuating psum * s -> bf16 tiles,
    DMA out column-major [96, T]; host transposes/un-permutes/upcasts.
"""

import sys
from contextlib import ExitStack

import numpy as np

for _p in ("/opt/trn_rl_repo",):
    if _p not in sys.path:
        sys.path.append(_p)

import ml_dtypes
import concourse.bass as bass
import concourse.bacc as bacc
import concourse.tile as tile
from concourse import mybir, bass_utils

F32 = mybir.dt.float32
BF16 = mybir.dt.bfloat16
FP8 = mybir.dt.float8e4
I16 = mybir.dt.int16
ALU = mybir.AluOpType
ACTF = mybir.ActivationFunctionType

V, D, R = 50257, 768, 16
SCALING = 32.0 / 16.0
N_CORES = 8
CPC = D // N_CORES          # columns per core = 96
W = CPC + R                 # 112 packed feature width
EF8 = 256.0                 # fp8 pre-scale for E and A in pass 1
VP = 50688                  # vocab padded: 99 groups of 512 rows
NG = VP // 512              # 99 row-groups (4 matmuls each)
GL = 9                      # groups per DMA macro-load (99 = 11 * 9)
PAIRS = ((V + 1) // 2 + 127) // 128 * 128  # 25216 pair rows
NTOK = 8 * 2048
CHUNK = 2048                # max tokens per dma_gather
HB = 512                    # tokens per matmul block (one PSUM bank)


def _chunk_sizes(n):
    """Split one parity bucket into gather chunks: multiples of HB, <=CHUNK."""
    out = []
    while n > 0:
        take = min(CHUNK, (min(n, CHUNK) + HB - 1) // HB * HB)
        out.append(take)
        n -= take
    return out


def _apply_drain_patch():
    """walrus in this container rejects >1 sem-wait on the Tile tail drain
    ("Too many sync wait commands"); split the waits across chained drains."""
    import concourse.tile as _tile_mod
    if getattr(_tile_mod.TileContext, "_drain_patch_applied", False):
        return

    def _drain_and_barrier(self, tick_clock, wait_clock):
        from concourse.tile import ScopedClock

        nc = self.nc
        drain_inst = nc.sync.drain()
        wait_clock.add_sem_waits(
            drain_inst.ins, ScopedClock({None: tick_clock.global_clock})
        )
        si = drain_inst.ins.sync_info
        if si is not None and si.on_wait and len(si.on_wait) > 1:
            waits = list(si.on_wait)
            del si.on_wait[1:]
            for w in waits[1:]:
                extra = nc.sync.drain()
                esi = extra.ins.sync_info
                if esi is None:
                    extra.ins.sync_info = mybir.SyncInfo(on_wait=[w], on_update=[])
                else:
                    esi.on_wait.append(w)
        nc.all_engine_barrier()
        assert self.sems is not None
        popped = nc._tile_sem_poison_stack.pop()
        assert popped is self._sem_poison
        nc.clear_and_free_semaphores(list(self.sems.allocated().values()))
        nc.all_engine_barrier()

    _tile_mod.TileContext._drain_and_barrier = _drain_and_barrier
    _tile_mod.TileContext._drain_patch_applied = True


_CACHED = {}
NREP = 1
VARIANT = "full"


def _build(nch):
    chunks = tuple(_build.chunk_parity)   # [(size, parity), ...]
    T = sum(s for s, _ in chunks)
    key = (NREP, VARIANT, chunks)
    if key in _CACHED:
        return _CACHED[key]
    _apply_drain_patch()

    nc = bacc.Bacc("TRN2", target_bir_lowering=False, debug=False)
    d_p1t = nc.dram_tensor("p1t", [VP // 4, 512], FP8, kind="ExternalInput").ap()
    d_gtab = nc.dram_tensor("gtab", [PAIRS, 256], BF16, kind="ExternalInput").ap()
    # sb96: rows 96:112 hold SCALING*B[:, cols] (f32); others zero
    d_sb96 = nc.dram_tensor("sb96", [128, CPC], F32, kind="ExternalInput").ap()
    # w2c: unscaled fused pass-2 stationary [[I_96], [SCALING*B]] bf16
    d_w2c = nc.dram_tensor("w2c", [W, CPC], BF16, kind="ExternalInput").ap()
    d_mag = nc.dram_tensor("magT", [1, CPC], F32, kind="ExternalInput").ap()
    d_idf = nc.dram_tensor("identf", [128, 128], F32, kind="ExternalInput").ap()
    d_i96 = nc.dram_tensor("i96", [CPC, CPC], F32, kind="ExternalInput").ap()
    # twoi96: rows 96:112 hold 2*I_16; others zero
    d_twoi = nc.dram_tensor("twoi96", [128, R], F32, kind="ExternalInput").ap()
    # ones16_96: rows 96:112 hold 1.0; others zero
    d_ones16 = nc.dram_tensor("ones16_96", [128, 1], F32, kind="ExternalInput").ap()
    d_ones = nc.dram_tensor("ones1", [1, 128], F32, kind="ExternalInput").ap()
    d_pidx = nc.dram_tensor("pidx", [128, T // 16], I16,
                            kind="ExternalInput").ap()
    # column-major output: padded-stream token j at column j, rows = 96 cols
    d_out = nc.dram_tensor("out", [CPC, T], BF16, kind="ExternalOutput").ap()

    with tile.TileContext(nc) as tc, ExitStack() as ctx:
        const = ctx.enter_context(tc.tile_pool(name="const", bufs=1))

        sb96_sb = const.tile([128, CPC], F32)
        nc.sync.dma_start(out=sb96_sb, in_=d_sb96)
        w2c_sb = const.tile([W, CPC], BF16)
        nc.sync.dma_start(out=w2c_sb, in_=d_w2c)
        mag_sb = const.tile([1, CPC], F32)
        nc.sync.dma_start(out=mag_sb, in_=d_mag)
        idf_sb = const.tile([128, 128], F32)
        nc.sync.dma_start(out=idf_sb, in_=d_idf)
        i96_sb = const.tile([CPC, CPC], F32)
        nc.sync.dma_start(out=i96_sb, in_=d_i96)
        twoi_sb = const.tile([128, R], F32)
        nc.sync.dma_start(out=twoi_sb, in_=d_twoi)
        ones16_sb = const.tile([128, 1], F32)
        nc.sync.dma_start(out=ones16_sb, in_=d_ones16)
        ones_sb = const.tile([1, 128], F32)
        nc.sync.dma_start(out=ones_sb, in_=d_ones)
        pidx_sb = const.tile([128, T // 16], I16)
        nc.sync.dma_start(out=pidx_sb, in_=d_pidx)

        def _dummy_out(rep_pool, src=None):
            outt0 = rep_pool.tile([CPC, 8], BF16)
            nc.vector.memset(outt0, 0.0)
            if src is not None:
                nc.vector.tensor_copy(out=outt0[0:1, 0:1], in_=src)
            nc.sync.dma_start(out=d_out[0:CPC, 0:8], in_=outt0)

        def _emit(chunk_parity, rep_pool, p1l, p1ps, sc, scps, p2g, p2ps, p2o):
            if VARIANT == "nop":
                _dummy_out(rep_pool)
                return

            # ---- pass 2 gathers: transposed 256B rows (issue first) ----
            # single_packet=True overflows the 64-desc/packet ring limit at
            # num_idxs>=1024 on the transpose path (device-fatal); keep False.
            gt = []
            n_gather = {"p2g1": 1}.get(VARIANT, len(chunks))
            if VARIANT not in ("p1", "p1mm", "p1dma"):
                coff = 0
                for c in range(n_gather):
                    size, par = chunks[c]
                    g = p2g.tile([128, 1, CHUNK], BF16, tag="g")
                    nc.gpsimd.dma_gather(
                        g[:, :, 0:size],
                        d_gtab[:, 128 * par : 128 * par + 128],
                        pidx_sb[:, coff // 16 : (coff + size) // 16],
                        num_idxs=size,
                        num_idxs_reg=size,
                        elem_size=128,
                        elem_step=256,
                        transpose=True,
                        single_packet=False,
                    )
                    gt.append(g)
                    coff += size
            if VARIANT in ("p2g", "p2g1"):
                outt0 = rep_pool.tile([CPC, 128], BF16)
                nc.scalar.copy(out=outt0, in_=gt[0][0:CPC, 0, 0:128])
                nc.sync.dma_start(out=d_out[0:CPC, 0:128], in_=outt0)
                return

            # ---- pass 1: self-Gram over vocab ----
            if VARIANT == "p2":
                sclv = sc.tile([CPC, 1], F32, tag="sclv")
                nc.vector.memset(sclv, 1.0)
            else:
                gram = p1ps.tile([W, W], F32, tag="gram")
                n_mm = NG * 4
                k = 0
                for i in range(NG // GL):
                    t = p1l.tile([128, GL, 4, 128], FP8)
                    nc.sync.dma_start(
                        out=t,
                        in_=d_p1t[i * GL * 128 : (i + 1) * GL * 128, :].rearrange(
                            "(g p) (j w) -> p g j w", p=128, j=4
                        ),
                    )
                    if VARIANT == "p1dma":
                        continue
                    for g in range(GL):
                        for j in range(4):
                            nc.tensor.matmul(
                                gram,
                                t[:, g, j, 0:W],
                                t[:, g, j, 0:W],
                                start=(k == 0),
                                stop=(k == n_mm - 1),
                            )
                            k += 1
                if VARIANT in ("p1mm", "p1dma"):
                    outt0 = rep_pool.tile([CPC, 8], BF16)
                    if VARIANT == "p1mm":
                        nc.scalar.copy(out=outt0, in_=gram[0:CPC, 0:8])
                    else:
                        nc.vector.memset(outt0, 0.0)
                    nc.sync.dma_start(out=d_out[0:CPC, 0:8], in_=outt0)
                    return

                # ---- sumsq (x65536) from Gram pieces ----
                gram_sb = sc.tile([W, W], F32, tag="gram_sb")
                nc.vector.tensor_copy(out=gram_sb, in_=gram)
                # t1[96, 1] = diag(E'E')
                dd = sc.tile([CPC, CPC], F32, tag="dd")
                nc.vector.tensor_tensor(
                    out=dd, in0=gram_sb[0:CPC, 0:CPC], in1=i96_sb, op=ALU.mult
                )
                t1 = sc.tile([CPC, 1], F32)
                nc.vector.reduce_sum(out=t1, in_=dd, axis=mybir.AxisListType.X)
                # u[16@96, 96] = A'A' @ sB + 2I @ M^T  (Gram slices in place)
                u_ps = scps.tile([128, CPC], F32, tag="chain")
                nc.tensor.matmul(u_ps[96:W, :], gram_sb[CPC:W, CPC:W],
                                 sb96_sb[96:W, :], start=True, stop=False,
                                 tile_position=(96, 96))
                nc.tensor.matmul(u_ps[96:W, :], twoi_sb[96:W, :],
                                 gram_sb[CPC:W, 0:CPC], start=False, stop=True,
                                 tile_position=(96, 96))
                vsb = sc.tile([128, CPC], F32, tag="vsb")
                nc.vector.tensor_tensor(out=vsb[96:W, :], in0=u_ps[96:W, :],
                                        in1=sb96_sb[96:W, :], op=ALU.mult)
                # ssT[1, 96] = ones16^T @ vsb + t1^T
                red_ps = scps.tile([128, CPC], F32, tag="chain")
                nc.tensor.matmul(red_ps[0:1, :], ones16_sb[96:W, :],
                                 vsb[96:W, :], start=True, stop=False,
                                 tile_position=(96, 0))
                nc.tensor.matmul(red_ps[0:1, :], t1, idf_sb[:CPC, :CPC],
                                 start=False, stop=True, is_transpose=True)
                ssT = sc.tile([1, CPC], F32)
                nc.vector.tensor_copy(out=ssT, in_=red_ps[0:1, :])

                # ---- sclT = 256*mag * rsqrt(ssT)  [1, 96] ----
                nrm = sc.tile([1, CPC], F32)
                nc.scalar.activation(nrm, ssT, ACTF.Sqrt)
                nc.vector.tensor_scalar(
                    out=nrm, in0=nrm, scalar1=EF8 * 1e-8, scalar2=None,
                    op0=ALU.max,
                )
                r0 = sc.tile([1, CPC], F32)
                nc.vector.reciprocal(out=r0, in_=nrm)
                sclT = sc.tile([1, CPC], F32)
                nc.vector.tensor_tensor(out=sclT, in0=r0, in1=mag_sb,
                                        op=ALU.mult)
                # transpose to per-partition [96, 1] for the evacuation scale
                tp_ps = scps.tile([128, CPC], F32, tag="chain")
                nc.tensor.matmul(tp_ps[0:CPC, 0:1], sclT, ones_sb[0:1, 0:1],
                                 is_transpose=True)
                sclv = sc.tile([CPC, 1], F32, tag="sclv")
                nc.vector.tensor_copy(out=sclv, in_=tp_ps[0:CPC, 0:1])

            if VARIANT == "p1":
                _dummy_out(rep_pool, sclv[0:1, 0:1])
                return

            # ---- pass 2: one fused matmul per 512-token block ----
            sclv_bc = bass.AP(
                tensor=sclv.tensor, offset=sclv.offset,
                ap=[list(sclv.ap[0]), [0, HB]],
            )
            coff = 0
            blk = 0
            for c, (size, _par) in enumerate(chunks):
                g = gt[c]
                nb = size // HB
                ot = p2o.tile([CPC, CHUNK // HB, HB], BF16, tag="ot")
                for h in range(nb):
                    ps = p2ps.tile([128, HB], F32, tag="ps")
                    nc.tensor.matmul(
                        ps[0:CPC, :], w2c_sb, g[0:W, 0, h * HB : (h + 1) * HB],
                        start=True, stop=True,
                    )
                    if blk % 2 == 0:
                        nc.vector.tensor_tensor(
                            out=ot[:, h, :], in0=ps[0:CPC, :], in1=sclv_bc,
                            op=ALU.mult,
                        )
                    else:
                        nc.scalar.activation(
                            ot[:, h, :], ps[0:CPC, :], ACTF.Copy, scale=sclv,
                        )
                    blk += 1
                nc.sync.dma_start(
                    out=d_out[:, coff : coff + size].rearrange(
                        "p (h f) -> p h f", h=nb
                    ),
                    in_=ot[:, 0:nb, :],
                )
                coff += size

        chunk_parity = _build.chunk_parity
        for _rep in range(NREP):
            if _rep:
                tc.strict_bb_all_engine_barrier()
            with (
                tc.tile_pool(name=f"rep{_rep}", bufs=1) as rep_pool,
                tc.tile_pool(name=f"p1l{_rep}", bufs=3) as p1l,
                tc.tile_pool(name=f"p1ps{_rep}", bufs=1, space="PSUM") as p1ps,
                tc.tile_pool(name=f"sc{_rep}", bufs=1) as sc,
                tc.tile_pool(name=f"scps{_rep}", bufs=1, space="PSUM") as scps,
                tc.tile_pool(name=f"p2g{_rep}", bufs=max(2, len(chunks))) as p2g,
                tc.tile_pool(name=f"p2ps{_rep}", bufs=6, space="PSUM") as p2ps,
                tc.tile_pool(name=f"p2o{_rep}", bufs=3) as p2o,
            ):
                _emit(chunk_parity, rep_pool, p1l, p1ps, sc, scps, p2g,
                      p2ps, p2o)

    nc.compile()
    _CACHED[key] = nc
    return nc


_build.chunk_parity = []


def _host_prep(inputs, embeddings, lora_a, lora_b, magnitude):
    E = np.asarray(embeddings, np.float32)
    A = np.asarray(lora_a, np.float32)
    B = np.asarray(lora_b, np.float32)
    mag = np.asarray(magnitude, np.float32)
    ids = np.asarray(inputs).astype(np.int64).reshape(-1)

    # ---- token parity bucketing, variable-size gather chunks ----
    even_pos = np.flatnonzero((ids & 1) == 0)
    odd_pos = np.flatnonzero((ids & 1) == 1)
    ne, no = len(even_pos), len(odd_pos)
    ce, co = _chunk_sizes(ne), _chunk_sizes(no)
    chunks = [(s, 0) for s in ce] + [(s, 1) for s in co]
    Se = sum(ce)
    T = Se + sum(co)
    perm = np.concatenate([even_pos, odd_pos])
    pair_ids = np.zeros(T, np.int64)
    pair_ids[:ne] = ids[even_pos] // 2
    pair_ids[Se : Se + no] = ids[odd_pos] // 2
    pidx_np = np.tile(
        pair_ids.astype(np.int16).reshape(T // 16, 16).T, (8, 1)
    ).copy()

    # ---- pass-1 fp8 table: [E*256 | A*256 | pad] vocab-major ----
    fp8 = ml_dtypes.float8_e4m3
    EAf = np.zeros((VP, 128), np.float32)
    EAf[:V, CPC : CPC + R] = EF8 * A

    # ---- pass-2 bf16 pair table ----
    Epad = np.zeros((2 * PAIRS, D), np.float32)
    Epad[:V] = E
    Apad = np.zeros((2 * PAIRS, R), np.float32)
    Apad[:V] = A

    idf_np = np.eye(128, dtype=np.float32)
    i96_np = np.eye(CPC, dtype=np.float32)
    twoi_np = np.zeros((128, R), np.float32)
    twoi_np[96:W, :] = 2.0 * np.eye(R, dtype=np.float32)
    ones16_np = np.zeros((128, 1), np.float32)
    ones16_np[96:W, :] = 1.0
    ones1_np = np.ones((1, 128), np.float32)

    in_maps = []
    for c in range(N_CORES):
        cols = slice(CPC * c, CPC * (c + 1))
        EAf[:V, 0:CPC] = EF8 * E[:, cols]
        p1t = EAf.astype(fp8).reshape(VP // 4, 512)

        ec = Epad[:, cols]
        gtab = np.zeros((PAIRS, 256), dtype=ml_dtypes.bfloat16)
        gtab[:, 0:CPC] = ec[0::2]
        gtab[:, CPC : CPC + R] = Apad[0::2]
        gtab[:, 128 : 128 + CPC] = ec[1::2]
        gtab[:, 128 + CPC : 128 + CPC + R] = Apad[1::2]

        sb96 = np.zeros((128, CPC), np.float32)
        sb96[96:W, :] = SCALING * B[:, cols]
        w2c = np.zeros((W, CPC), np.float32)
        w2c[0:CPC, :] = i96_np
        w2c[96:W, :] = SCALING * B[:, cols]
        in_maps.append(
            {
                "p1t": p1t,
                "gtab": gtab,
                "sb96": sb96,
                "w2c": w2c.astype(ml_dtypes.bfloat16),
                "magT": np.ascontiguousarray(EF8 * mag[cols])[None, :],
                "identf": idf_np,
                "i96": i96_np,
                "twoi96": twoi_np,
                "ones16_96": ones16_np,
                "ones1": ones1_np,
                "pidx": pidx_np,
            }
        )
    return in_maps, perm, ne, no, Se, len(chunks), chunks


def kernel(inputs, embeddings, lora_a, lora_b, magnitude, _trace=False):
    in_maps, perm, ne, no, Se, nch, chunks = _host_prep(
        inputs, embeddings, lora_a, lora_b, magnitude
    )
    _build.chunk_parity = chunks
    nc = _build(nch)
    res = bass_utils.run_bass_kernel_spmd(
        nc, in_maps, core_ids=list(range(N_CORES)), trace=_trace
    )
    shp = np.asarray(inputs).shape
    out = np.empty((NTOK, D), np.float32)
    valid = np.concatenate([np.arange(ne), Se + np.arange(no)])
    for c in range(N_CORES):
        blk = np.asarray(res.results[c]["out"], dtype=np.float32)
        # [96, nch*1024] column-major -> token-ordered [nch*1024, 96]
        out[perm, CPC * c : CPC * (c + 1)] = blk[:, valid].T
    out = out.reshape(shp + (D,))
    if _trace:
        return out, res
    return out
